# revision 21
# baseline (speedup 1.0000x reference)
"""Trainium2 Bass kernel for nn_Attention_5583457485032.

Computes, for each of 2 heads (W[i] is (256,256)), iterated twice:
    temp = mean(xi, 0);  h = tanh(temp @ Wi);  s = xi @ h.T
    att = sigmoid(s / max(|s|, 1e-12))   # == sigmoid(sign(s))
    out = att.T @ xi;  xi = xi * att
and returns concat of head outputs, shape (1, 512).

Key restructuring (algebraically exact):
  - round-2 mean(xi) == out1 / N, so xi never needs materializing
  - att == sigmoid(sign(s)) since s/max(|s|,eps) == sign(s) for |s|>eps
  - per head: out2 = sum_i att1_i att2_i x_i with
      s1_i = x_i . h1, att1 = sigm(sign(s1)), out1 = sum att1_i x_i
      s2 sign == sign(x_i . h2) (att1 > 0)
Distribution: shard x row-wise over 8 cores; colsum and the (2,256)
out1 partials go through AllReduce; final out2 partials are summed on
host. Both heads are batched into the same matmuls.

v1: single HBM read of x. DMA casts f32->bf16 (SWDGE) into a resident
SBUF copy x_nat; pass A also builds a PE-transposed resident copy xts
(per 128-row tile) while computing the colsum. Passes B and C then run
entirely from SBUF: scores via xts-stationary matmuls, per-8-tile
batched sign/sigmoid on ScalarE, weighted column-sum accumulation into
PSUM. All big matmuls are bf16 (single-pass on the PE, double-pumped).
"""

import os
import numpy as np

N_CORES = 8
N_TOTAL = 200000
D = 256
H = 2
P = 128
TPC = 8   # tiles per DMA chunk
G = 8     # tiles per sign/sigmoid group


def build_kernel(n_rows, n_cores, n_total=None):
    import concourse.bass as bass
    import concourse.mybir as mybir

    F32 = mybir.dt.float32
    BF16 = mybir.dt.bfloat16
    AF = mybir.ActivationFunctionType
    ALU = mybir.AluOpType

    if n_total is None:
        n_total = n_rows * n_cores

    T = (n_rows + P - 1) // P
    n_chunks = (T + TPC - 1) // TPC
    chunk_tiles = [list(range(c * TPC, min(T, (c + 1) * TPC))) for c in range(n_chunks)]
    n_groups = (T + G - 1) // G
    group_tiles = [list(range(g * G, min(T, (g + 1) * G))) for g in range(n_groups)]

    def rows_of(t):
        return min(P, n_rows - t * P)

    nc = bass.Bass()
    x_ext = nc.declare_dram_parameter("x", [n_rows, D], F32, isOutput=False)
    w_ext = nc.declare_dram_parameter("W", [H, D, D], F32, isOutput=False)
    out_ext = nc.declare_dram_parameter("out", [H, D], F32, isOutput=True)

    cs_dram = nc.dram_tensor("cs_dram", [P, 2], F32)
    cs_ar = nc.dram_tensor("cs_ar", [P, 2], F32)
    o1_dram = nc.dram_tensor("o1_dram", [H, D], F32)
    o1_ar = nc.dram_tensor("o1_ar", [H, D], F32)

    sb = nc.alloc_sbuf_tensor
    x_nat = sb("x_nat", [P, T * D], BF16)      # tile t at cols [t*D,(t+1)*D)
    xts = sb("xts", [P, T * D], BF16)          # tile t: [t*D + oc*P + row]
    ones_f = sb("ones_f", [P, P], F32)
    iden_f = sb("iden_f", [P, P], F32)
    ones_b = sb("ones_b", [P, P], BF16)
    iden_b = sb("iden_b", [P, P], BF16)
    wsb = sb("wsb", [P, H * 2 * 2 * P], F32)   # (h,dc,oc) at col ((h*2+dc)*2+oc)*128
    o1_sb = sb("o1_sb", [H, D], F32)           # also reused for final out2 staging
    o1r_sb = sb("o1r_sb", [H, D], F32)
    cs_col = sb("cs_col", [P, 2], F32)         # [:, dc]
    cacc = sb("cacc", [P, 2 * n_chunks], F32)  # slot [oc*n_chunks + c]
    cs2_col = sb("cs2_col", [P, 2 * H], F32)   # [:, dc*H + h]
    hcol = sb("hcol", [P, 2 * H], BF16)        # [:, oc*H + h]
    sgn = [sb(f"sgn{b}", [P, G * H], F32) for b in range(2)]
    att1 = sb("att1", [P, T * H], BF16)        # tile t at cols [t*H, t*H+H)
    att2 = [sb(f"att2{b}", [P, G * H], BF16) for b in range(2)]
    wv = [sb(f"wv{b}", [P, G * H], BF16) for b in range(2)]

    ps = nc.alloc_psum_tensor
    xtp = [ps(f"xtp{b}", [P, 2 * P], BF16) for b in range(2)]
    spsum = [ps(f"spsum{b}", [P, G * H], F32) for b in range(2)]
    outacc = ps("outacc", [H, D], F32)
    cst = ps("cst", [P, 2 * H], F32)
    ht = ps("ht", [P, 2 * H], F32)

    sems = {k: nc.alloc_semaphore(k) for k in
            ("dma_w", "dma_x0", "dma_x1", "dma_m", "pe", "act", "dve", "cc",
             "poolc")}

    ENGS = ("sp", "pe", "act", "dve", "pool")

    class Sched:
        def __init__(self, plan=None):
            self.plan = plan
            self.ctr = {k: 0 for k in sems}
            self.ev = {} if plan is None else plan
            self.ops = {e: [] for e in ENGS}
            self.seen = {e: {} for e in ENGS}

        def inst(self, eng, sem, thunk, key=None, step=1):
            self.ctr[sem] += step
            v = self.ctr[sem]
            if self.plan is None:
                if key is not None:
                    assert key not in self.ev, key
                    self.ev[key] = (sem, v)
            else:
                if key is not None:
                    assert self.ev[key] == (sem, v), (key, self.ev[key], sem, v)
                self.ops[eng].append(("i", thunk, sem, step))
            return v

        def wait(self, eng, key):
            if self.plan is None:
                return
            sem, v = self.ev[key]
            if v <= 0 or self.seen[eng].get(sem, 0) >= v:
                return
            self.seen[eng][sem] = v
            self.ops[eng].append(("w", sem, v))

    def chunk_load(S, c):
        """SWDGE (gpsimd) DMA with f32->bf16 cast straight into x_nat."""
        r0 = c * TPC * P
        r1 = min(n_rows, (c + 1) * TPC * P)
        rows = r1 - r0
        nt = rows // P
        tail = rows - nt * P
        sem = f"dma_x{c % 2}"
        key = ("dma", "load", c)
        if nt:
            def f(c=c, r0=r0, nt=nt):
                src = x_ext[r0:r0 + nt * P, :].rearrange("(n p) m -> p n m", p=P)
                dst = x_nat[:, c * TPC * D:c * TPC * D + nt * D]
                return nc.gpsimd.dma_start(out=dst, in_=src)
            S.inst("pool", sem, f, step=16, key=None if tail else key)
        if tail:
            def f2(c=c, r0=r0, nt=nt, tail=tail):
                base = (c * TPC + nt) * D
                return nc.gpsimd.dma_start(
                    out=x_nat[0:tail, base:base + D],
                    in_=x_ext[r0 + nt * P:r0 + nt * P + tail, :])
            S.inst("pool", sem, f2, step=16, key=key)

    def h_phase(S, tag, cc_key, col_sb, col_idx):
        """AR result -> column layout -> 8 W-matmuls (f32) + tanh -> hcol.
        h1: the AR already runs in column layout (128,2) -> no transposes.
        h2: AR output is the (2,256) out1 row vector -> PE transposes."""
        S.wait("sp", cc_key)
        if tag == "h1":
            S.inst("sp", "dma_m",
                   lambda: nc.sync.dma_start(out=cs_col[:, :], in_=cs_ar[:, :]),
                   step=16, key=("dma", "h1_in"))
        else:
            S.inst("sp", "dma_m",
                   lambda: nc.sync.dma_start(out=o1r_sb[0:H, :], in_=o1_ar[:, :]),
                   step=16, key=("dma", "h2_in"))
            S.wait("pe", ("dma", "h2_in"))
            S.wait("pe", ("dve", "const"))
            for dc in range(2):
                S.inst("pe", "pe",
                       lambda dc=dc:
                       nc.tensor.transpose(
                           cst[:, dc * H:(dc + 1) * H],
                           o1r_sb[0:H, dc * P:(dc + 1) * P],
                           iden_f[0:H, 0:H]),
                       key=("pe", "h2_tr") if dc == 1 else None)
            S.wait("dve", ("pe", "h2_tr"))
            S.inst("dve", "dve",
                   lambda: nc.vector.tensor_copy(cs2_col[:, :], cst[:, :]),
                   key=("dve", "h2_col"))
        S.wait("pe", ("dma", "W"))
        if tag == "h1":
            S.wait("pe", ("dma", "h1_in"))
        else:
            S.wait("pe", ("dve", "h2_col"))
        for h in range(H):
            for oc in range(2):
                for dc in range(2):
                    widx = (h * 2 + dc) * 2 + oc
                    S.inst("pe", "pe",
                           lambda h=h, oc=oc, dc=dc, widx=widx, col_sb=col_sb:
                           nc.tensor.matmul(
                               ht[:, oc * H + h:oc * H + h + 1],
                               wsb[:, widx * P:(widx + 1) * P],
                               col_sb[:, col_idx(dc, h):col_idx(dc, h) + 1],
                               start=(dc == 0), stop=(dc == 1),
                               skip_group_check=True),
                           key=("pe", tag + "_mm")
                           if (h, oc, dc) == (H - 1, 1, 1) else None)
        S.wait("act", ("pe", tag + "_mm"))
        S.inst("act", "act",
               lambda: nc.scalar.activation(
                   hcol[:, :], ht[:, :], AF.Tanh, scale=1.0 / float(n_total)),
               key=("act", tag))

    def pass_bc(S, tag):
        """Per-tile scores+weighted-sum from resident SBUF copies.
        ACT sign/sigmoid batched per group of G tiles."""
        is_c = tag == "C"
        htag = "h1" if tag == "B" else "h2"

        def grp(t):
            return t // G

        def mms_tile(t):
            g = grp(t)
            b = g % 2
            r = rows_of(t)
            col = (t - g * G) * H
            if t == 0:
                S.wait("pe", ("act", htag))
                if tag == "B":
                    # all resident xts copies must have landed (the ACT-side
                    # ones are covered transitively through h1; wait on the
                    # last DVE-side copy explicitly)
                    last_dve = T - 1 if (T - 1) % 2 == 0 else T - 2
                    if last_dve >= 0:
                        S.wait("pe", ("cp", "A_copy", last_dve))
            if g >= 2 and t == group_tiles[g][0]:
                S.wait("pe", ("act", tag + "_sig", g - 2))
            for oc in range(2):
                S.inst("pe", "pe",
                       lambda t=t, b=b, r=r, col=col, oc=oc:
                       nc.tensor.matmul(
                           spsum[b][0:r, col:col + H],
                           xts[:, t * D + oc * P:t * D + oc * P + r],
                           hcol[:, oc * H:(oc + 1) * H],
                           start=(oc == 0), stop=(oc == 1),
                           skip_group_check=True),
                       key=("pe", tag + "_mmS", t) if oc == 1 else None)

        def mmout_tile(t):
            g = grp(t)
            r = rows_of(t)
            if is_c:
                S.wait("pe", ("dve", "C_w", g))
                lhs = lambda t=t, r=r, g=g: wv[g % 2][0:r, (t - g * G) * H:(t - g * G) * H + H]
            else:
                S.wait("pe", ("act", "B_sig", g))
                lhs = lambda t=t, r=r: att1[0:r, t * H:(t + 1) * H]
            S.inst("pe", "pe",
                   lambda t=t, r=r, lhs=lhs:
                   nc.tensor.matmul(
                       outacc[:, :],
                       lhs(),
                       x_nat[0:r, t * D:(t + 1) * D],
                       start=(t == 0), stop=(t == T - 1),
                       skip_group_check=True),
                   key=("pe", tag + "_mmOut", t))

        def act_group(g):
            b = g % 2
            tiles = group_tiles[g]
            ncols = len(tiles) * H
            rmax = rows_of(tiles[0])
            S.wait("act", ("pe", tag + "_mmS", tiles[-1]))
            if g >= 2:
                S.wait("act", ("act", tag + "_sig", g - 2))
                if is_c:
                    S.wait("act", ("dve", "C_w", g - 2))
            S.inst("act", "act",
                   lambda g=g, b=b, rmax=rmax, ncols=ncols:
                   nc.scalar.activation(
                       sgn[b][0:rmax, 0:ncols], spsum[b][0:rmax, 0:ncols],
                       AF.Sign),
                   key=("act", tag + "_sgn", g))
            S.wait("act", ("act", tag + "_sgn", g))
            if is_c:
                dst = lambda g=g, b=b, rmax=rmax, ncols=ncols: att2[b][0:rmax, 0:ncols]
            else:
                dst = lambda g=g, rmax=rmax, ncols=ncols: \
                    att1[0:rmax, g * G * H:g * G * H + ncols]
            S.inst("act", "act",
                   lambda g=g, b=b, rmax=rmax, ncols=ncols, dst=dst:
                   nc.scalar.activation(dst(), sgn[b][0:rmax, 0:ncols],
                                        AF.Sigmoid),
                   key=("act", tag + "_sig", g))

        def dve_w(g):
            b = g % 2
            tiles = group_tiles[g]
            ncols = len(tiles) * H
            rmax = rows_of(tiles[0])
            S.wait("dve", ("act", "C_sig", g))
            S.inst("dve", "dve",
                   lambda g=g, b=b, rmax=rmax, ncols=ncols:
                   nc.vector.tensor_mul(
                       wv[b][0:rmax, 0:ncols], att2[b][0:rmax, 0:ncols],
                       att1[0:rmax, g * G * H:g * G * H + ncols]),
                   key=("dve", "C_w", g))

        # PE stream: scores for group g+1 interleave with mmOut of group g
        for t in group_tiles[0]:
            mms_tile(t)
        for g in range(n_groups):
            if g + 1 < n_groups:
                for t in group_tiles[g + 1]:
                    mms_tile(t)
            for t in group_tiles[g]:
                mmout_tile(t)
        # ACT stream
        for g in range(n_groups):
            act_group(g)
        # DVE stream
        if is_c:
            for g in range(n_groups):
                dve_w(g)

    def col_idx_h1(dc, h):
        return dc

    def col_idx_h2(dc, h):
        return dc * H + h

    def sched(S):
        # ---- preamble: zero spsum tails (DVE; partial tiles leave rows
        # uninitialized and the grouped sign/sigmoid reads full rectangles)
        S.inst("dve", "dve", lambda: nc.vector.memset(spsum[0].ap(), 0.0))
        S.inst("dve", "dve", lambda: nc.vector.memset(spsum[1].ap(), 0.0))
        # ---- preamble: constants (gpsimd), W loads (sync) ----
        S.inst("pool", "poolc", lambda: nc.gpsimd.memset(ones_f.ap(), 1.0),
               key=("dve", "ones"))
        S.inst("pool", "poolc", lambda: nc.gpsimd.memset(ones_b.ap(), 1.0),
               key=("dve", "ones_b"))
        S.wait("pool", ("dve", "ones"))
        S.wait("pool", ("dve", "ones_b"))
        S.inst("pool", "poolc",
               lambda: nc.gpsimd.affine_select(
                   iden_f.ap(), ones_f.ap(), pattern=[[-1, P]],
                   compare_op=ALU.is_equal, fill=0.0, base=0,
                   channel_multiplier=1),
               key=("dve", "iden_f"))
        S.inst("pool", "poolc",
               lambda: nc.gpsimd.affine_select(
                   iden_b.ap(), ones_b.ap(), pattern=[[-1, P]],
                   compare_op=ALU.is_equal, fill=0.0, base=0,
                   channel_multiplier=1),
               key=("dve", "const"))
        for h in range(H):
            for dc in range(2):
                for oc in range(2):
                    widx = (h * 2 + dc) * 2 + oc
                    S.inst("sp", "dma_w",
                           lambda h=h, dc=dc, oc=oc, widx=widx:
                           nc.sync.dma_start(
                               out=wsb[:, widx * P:(widx + 1) * P],
                               in_=w_ext[h, dc * P:(dc + 1) * P, oc * P:(oc + 1) * P]),
                           step=16,
                           key=("dma", "W") if widx == H * 4 - 1 else None)

        # ---- phase A: chunk loads (pool, SWDGE cast) gated 2-deep ----
        for c in range(n_chunks):
            if c >= 2:
                S.wait("pool", ("pe", "A_chunk", c - 2))
            chunk_load(S, c)
        # ---- phase A: per tile 2 PE transposes; plain copies xtp->xts on
        # DVE (even tiles) / ACT (odd tiles); colsum via per-chunk 3D-AP
        # reduces over the resident xts (DVE), accumulated in cacc.
        if T * P != n_rows:
            # zero the tail tile's unwritten xts columns so chunk reduces
            # can read full rectangles
            tt = T - 1
            r = rows_of(tt)
            for oc in range(2):
                S.inst("dve", "dve",
                       lambda tt=tt, r=r, oc=oc:
                       nc.vector.memset(
                           xts[:, tt * D + oc * P + r:tt * D + (oc + 1) * P], 0.0))
        for c in range(n_chunks):
            S.wait("pe", ("dma", "load", c))
            if c == 0:
                S.wait("pe", ("dve", "const"))
            for t in chunk_tiles[c]:
                r = rows_of(t)
                if t >= 2:
                    S.wait("pe", ("cp", "A_copy", t - 2))
                for oc in range(2):
                    S.inst("pe", "pe",
                           lambda t=t, r=r, oc=oc:
                           nc.tensor.transpose(
                               xtp[t % 2][:, oc * P:oc * P + r],
                               x_nat[0:r, t * D + oc * P:t * D + (oc + 1) * P],
                               iden_b[0:r, 0:r]),
                           key=("pe", "A_tr", t) if oc == 1 else None)
                if t == chunk_tiles[c][-1]:
                    if S.plan is None:
                        S.ev[("pe", "A_chunk", c)] = S.ev[("pe", "A_tr", t)]
            # copies for this chunk
            for t in chunk_tiles[c]:
                r = rows_of(t)
                eng, sem = ("dve", "dve") if t % 2 == 0 else ("act", "act")
                S.wait(eng, ("pe", "A_tr", t))
                for oc in range(2):
                    if t % 2 == 0:
                        S.inst(eng, sem,
                               lambda t=t, r=r, oc=oc:
                               nc.vector.tensor_copy(
                                   xts[:, t * D + oc * P:t * D + oc * P + r],
                                   xtp[t % 2][:, oc * P:oc * P + r]),
                               key=("cp", "A_copy", t) if oc == 1 else None)
                    else:
                        S.inst(eng, sem,
                               lambda t=t, r=r, oc=oc:
                               nc.scalar.copy(
                                   xts[:, t * D + oc * P:t * D + oc * P + r],
                                   xtp[t % 2][:, oc * P:oc * P + r]),
                               key=("cp", "A_copy", t) if oc == 1 else None)
            # chunk colsum partials from resident xts (both halves)
            last_odd_c = max((t for t in chunk_tiles[c] if t % 2 == 1),
                             default=None)
            last_even_c = max((t for t in chunk_tiles[c] if t % 2 == 0),
                              default=None)
            if last_odd_c is not None:
                S.wait("dve", ("cp", "A_copy", last_odd_c))
            if last_even_c is not None:
                S.wait("dve", ("cp", "A_copy", last_even_c))
            base = chunk_tiles[c][0] * D
            nt = len(chunk_tiles[c])
            for oc in range(2):
                S.inst("dve", "dve",
                       lambda c=c, base=base, nt=nt, oc=oc:
                       nc.vector.tensor_reduce(
                           cacc[:, oc * n_chunks + c:oc * n_chunks + c + 1],
                           xts.ap().rearrange("p (t m) -> p t m", m=D)
                               [:, chunk_tiles[c][0]:chunk_tiles[c][0] + nt,
                                oc * P:(oc + 1) * P],
                           axis=mybir.AxisListType.XY,
                           op=ALU.add),
                       key=("dve", "A_red", c) if oc == 1 else None)
        # final colsum reduction (cacc -> cs_col)
        S.wait("dve", ("dve", "A_red", n_chunks - 1))
        for dc in range(2):
            S.inst("dve", "dve",
                   lambda dc=dc:
                   nc.vector.reduce_sum(
                       cs_col[:, dc:dc + 1],
                       cacc[:, dc * n_chunks:dc * n_chunks + n_chunks],
                       axis=mybir.AxisListType.X),
                   key=("dve", "cs_red") if dc == 1 else None)
        # colsum -> AR
        S.wait("sp", ("dve", "cs_red"))
        S.inst("sp", "dma_m",
               lambda: nc.sync.dma_start(out=cs_dram[:, :], in_=cs_col[:, :]),
               step=16, key=("dma", "cs_out"))
        S.wait("pool", ("dma", "cs_out"))
        S.inst("pool", "cc",
               lambda: nc.gpsimd.collective_compute(
                   "AllReduce", mybir.AluOpType.add,
                   replica_groups=[list(range(n_cores))],
                   ins=[cs_dram[:, :]], outs=[cs_ar[:, :]]),
               key=("cc", "ar1"))
        h_phase(S, "h1", ("cc", "ar1"), cs_col, col_idx_h1)
        # ---- phase B ----
        pass_bc(S, "B")
        # out1 -> AR
        S.wait("act", ("pe", "B_mmOut", T - 1))
        S.inst("act", "act",
               lambda: nc.scalar.copy(o1_sb[0:H, :], outacc[:, :]),
               key=("act", "o1_copy"))
        S.wait("sp", ("act", "o1_copy"))
        S.inst("sp", "dma_m",
               lambda: nc.sync.dma_start(out=o1_dram[:, :], in_=o1_sb[0:H, :]),
               step=16, key=("dma", "o1_out"))
        S.wait("pool", ("dma", "o1_out"))
        S.inst("pool", "cc",
               lambda: nc.gpsimd.collective_compute(
                   "AllReduce", mybir.AluOpType.add,
                   replica_groups=[list(range(n_cores))],
                   ins=[o1_dram[:, :]], outs=[o1_ar[:, :]]),
               key=("cc", "ar2"))
        h_phase(S, "h2", ("cc", "ar2"), cs2_col, col_idx_h2)
        # ---- phase C ----
        pass_bc(S, "C")
        # final out (stage through o1_sb)
        S.wait("act", ("pe", "C_mmOut", T - 1))
        S.inst("act", "act",
               lambda: nc.scalar.copy(o1_sb[0:H, :], outacc[:, :]),
               key=("act", "out_copy"))
        S.wait("sp", ("act", "out_copy"))
        S.inst("sp", "dma_m",
               lambda: nc.sync.dma_start(out=out_ext[:, :], in_=o1_sb[0:H, :]),
               step=16, key=("dma", "out_final"))
        S.wait("sp", ("dma", "out_final"))

    plan = Sched()
    sched(plan)
    emit = Sched(plan.ev)
    sched(emit)

    eng_map = {
        "sp": nc.sync, "pe": nc.tensor, "act": nc.scalar,
        "dve": nc.vector, "pool": nc.gpsimd,
    }

    def run_ops(eng_name):
        eng = eng_map[eng_name]
        def body(_engine):
            for op in emit.ops[eng_name]:
                if op[0] == "w":
                    _, sem, v = op
                    eng.wait_ge(sems[sem], v)
                else:
                    _, thunk, sem, step = op
                    bi = thunk()
                    bi.then_inc(sems[sem], step)
        return body

    with nc.Block() as block:
        block.sync(run_ops("sp"))
        block.gpsimd(run_ops("pool"))
        block.tensor(run_ops("pe"))
        block.scalar(run_ops("act"))
        block.vector(run_ops("dve"))

    return nc


_NC_CACHE = {}


def _get_nc(n_rows, n_cores, n_total):
    key = (n_rows, n_cores, n_total)
    if key not in _NC_CACHE:
        _NC_CACHE[key] = build_kernel(n_rows, n_cores, n_total)
    return _NC_CACHE[key]


def kernel(x, W):
    from concourse.bass_utils import run_bass_kernel_spmd

    x = np.ascontiguousarray(np.asarray(x, dtype=np.float32))
    W = np.ascontiguousarray(np.asarray(W, dtype=np.float32))
    n, d = x.shape
    assert n % N_CORES == 0 and d == D
    n_rows = n // N_CORES

    nc = _get_nc(n_rows, N_CORES, n)
    in_maps = [
        {"x": x[i * n_rows:(i + 1) * n_rows], "W": W} for i in range(N_CORES)
    ]
    res = run_bass_kernel_spmd(nc, in_maps, core_ids=list(range(N_CORES)))
    total = np.zeros((H, D), dtype=np.float64)
    for i in range(N_CORES):
        total += res.results[i]["out"].astype(np.float64)
    return total.astype(np.float32).reshape(1, H * D)


if __name__ == "__main__":
    rng = np.random.default_rng(0)
    x = rng.standard_normal((N_TOTAL, D)).astype(np.float32)
    W = (rng.standard_normal((H, D, D)) * np.sqrt(2.0 / (D + D))).astype(np.float32)
    out = kernel(x=x, W=W)
    print(out.shape, out[0, :4])


# revision 22
# speedup vs baseline: 1.0946x; 1.0946x over previous
"""Trainium2 Bass kernel for nn_Attention_5583457485032.

Computes, for each of 2 heads (W[i] is (256,256)), iterated twice:
    temp = mean(xi, 0);  h = tanh(temp @ Wi);  s = xi @ h.T
    att = sigmoid(s / max(|s|, 1e-12))   # == sigmoid(sign(s))
    out = att.T @ xi;  xi = xi * att
and returns concat of head outputs, shape (1, 512).

Key restructuring (algebraically exact):
  - round-2 mean(xi) == out1 / N, so xi never needs materializing
  - att == sigmoid(sign(s)) since s/max(|s|,eps) == sign(s) for |s|>eps
  - per head: out2 = sum_i att1_i att2_i x_i with
      s1_i = x_i . h1, att1 = sigm(sign(s1)), out1 = sum att1_i x_i
      s2 sign == sign(x_i . h2) (att1 > 0)
Distribution: shard x row-wise over 8 cores; colsum and the (2,256)
out1 partials go through AllReduce; final out2 partials are summed on
host. Both heads are batched into the same matmuls.

v1: single HBM read of x. DMA casts f32->bf16 (SWDGE) into a resident
SBUF copy x_nat; pass A also builds a PE-transposed resident copy xts
(per 128-row tile) while computing the colsum. Passes B and C then run
entirely from SBUF: scores via xts-stationary matmuls, per-8-tile
batched sign/sigmoid on ScalarE, weighted column-sum accumulation into
PSUM. All big matmuls are bf16 (single-pass on the PE, double-pumped).
"""

import os
import numpy as np

N_CORES = 8
N_TOTAL = 200000
D = 256
H = 2
P = 128
TPC = 8   # tiles per DMA chunk
G = 8     # tiles per sign/sigmoid group


def build_kernel(n_rows, n_cores, n_total=None):
    import concourse.bass as bass
    import concourse.mybir as mybir

    F32 = mybir.dt.float32
    BF16 = mybir.dt.bfloat16
    AF = mybir.ActivationFunctionType
    ALU = mybir.AluOpType

    if n_total is None:
        n_total = n_rows * n_cores

    T = (n_rows + P - 1) // P
    n_chunks = (T + TPC - 1) // TPC
    chunk_tiles = [list(range(c * TPC, min(T, (c + 1) * TPC))) for c in range(n_chunks)]
    n_groups = (T + G - 1) // G
    group_tiles = [list(range(g * G, min(T, (g + 1) * G))) for g in range(n_groups)]

    def rows_of(t):
        return min(P, n_rows - t * P)

    nc = bass.Bass()
    x_ext = nc.declare_dram_parameter("x", [n_rows, D], F32, isOutput=False)
    w_ext = nc.declare_dram_parameter("W", [H, D, D], F32, isOutput=False)
    out_ext = nc.declare_dram_parameter("out", [H, D], F32, isOutput=True)

    cs_dram = nc.dram_tensor("cs_dram", [P, 2], F32)
    cs_ar = nc.dram_tensor("cs_ar", [P, 2], F32)
    o1_dram = nc.dram_tensor("o1_dram", [H, D], F32)
    o1_ar = nc.dram_tensor("o1_ar", [H, D], F32)

    sb = nc.alloc_sbuf_tensor
    x_nat = sb("x_nat", [P, T * D], BF16)      # tile t at cols [t*D,(t+1)*D)
    xts = sb("xts", [P, T * D], BF16)          # tile t: [t*D + oc*P + row]
    ones_f = sb("ones_f", [P, P], F32)
    iden_f = sb("iden_f", [P, P], F32)
    ones_b = sb("ones_b", [P, P], BF16)
    iden_b = sb("iden_b", [P, P], BF16)
    wsb = sb("wsb", [P, H * 2 * 2 * P], F32)   # (h,dc,oc) at col ((h*2+dc)*2+oc)*128
    o1_sb = sb("o1_sb", [H, D], F32)           # also reused for final out2 staging
    o1r_sb = sb("o1r_sb", [H, D], F32)
    cs_col = sb("cs_col", [P, 2], F32)         # [:, dc]
    cacc = sb("cacc", [P, 2 * n_chunks], F32)  # slot [oc*n_chunks + c]
    cs2_col = sb("cs2_col", [P, 2 * H], F32)   # [:, dc*H + h]
    hcol = sb("hcol", [P, 2 * H], BF16)        # [:, oc*H + h]
    sgn = [sb(f"sgn{b}", [P, G * H], F32) for b in range(2)]
    att1 = sb("att1", [P, T * H], BF16)        # tile t at cols [t*H, t*H+H)
    att2 = [sb(f"att2{b}", [P, G * H], BF16) for b in range(2)]
    wv = [sb(f"wv{b}", [P, G * H], BF16) for b in range(2)]

    ps = nc.alloc_psum_tensor
    xtp = [ps(f"xtp{b}", [P, 2 * P], BF16) for b in range(4)]
    spsum = [ps(f"spsum{b}", [P, G * H], F32) for b in range(2)]
    outacc = ps("outacc", [H, D], F32)
    miscp = ps("miscp", [P, 4 * H], F32)
    cst = miscp.ap()[:, 0:2 * H]
    ht = miscp.ap()[:, 2 * H:4 * H]

    sems = {k: nc.alloc_semaphore(k) for k in
            ("dma_w", "dma_x0", "dma_x1", "dma_m", "pe", "act", "dve", "cc",
             "poolc")}

    ENGS = ("sp", "pe", "act", "dve", "pool")

    class Sched:
        def __init__(self, plan=None):
            self.plan = plan
            self.ctr = {k: 0 for k in sems}
            self.ev = {} if plan is None else plan
            self.ops = {e: [] for e in ENGS}
            self.seen = {e: {} for e in ENGS}

        def inst(self, eng, sem, thunk, key=None, step=1):
            self.ctr[sem] += step
            v = self.ctr[sem]
            if self.plan is None:
                if key is not None:
                    assert key not in self.ev, key
                    self.ev[key] = (sem, v)
            else:
                if key is not None:
                    assert self.ev[key] == (sem, v), (key, self.ev[key], sem, v)
                self.ops[eng].append(("i", thunk, sem, step))
            return v

        def wait(self, eng, key):
            if self.plan is None:
                return
            sem, v = self.ev[key]
            if v <= 0 or self.seen[eng].get(sem, 0) >= v:
                return
            self.seen[eng][sem] = v
            self.ops[eng].append(("w", sem, v))

    def chunk_load(S, c):
        """SWDGE (gpsimd) DMA with f32->bf16 cast straight into x_nat."""
        r0 = c * TPC * P
        r1 = min(n_rows, (c + 1) * TPC * P)
        rows = r1 - r0
        nt = rows // P
        tail = rows - nt * P
        sem = f"dma_x{c % 2}"
        key = ("dma", "load", c)
        if nt:
            def f(c=c, r0=r0, nt=nt):
                src = x_ext[r0:r0 + nt * P, :].rearrange("(n p) m -> p n m", p=P)
                dst = x_nat[:, c * TPC * D:c * TPC * D + nt * D]
                return nc.gpsimd.dma_start(out=dst, in_=src)
            S.inst("pool", sem, f, step=16, key=None if tail else key)
        if tail:
            def f2(c=c, r0=r0, nt=nt, tail=tail):
                base = (c * TPC + nt) * D
                return nc.gpsimd.dma_start(
                    out=x_nat[0:tail, base:base + D],
                    in_=x_ext[r0 + nt * P:r0 + nt * P + tail, :])
            S.inst("pool", sem, f2, step=16, key=key)

    def h_phase(S, tag, cc_key, col_sb, col_idx):
        """AR result -> column layout -> 8 W-matmuls (f32) + tanh -> hcol.
        h1: the AR already runs in column layout (128,2) -> no transposes.
        h2: AR output is the (2,256) out1 row vector -> PE transposes."""
        S.wait("sp", cc_key)
        if tag == "h1":
            S.inst("sp", "dma_m",
                   lambda: nc.sync.dma_start(out=cs_col[:, :], in_=cs_ar[:, :]),
                   step=16, key=("dma", "h1_in"))
        else:
            S.inst("sp", "dma_m",
                   lambda: nc.sync.dma_start(out=o1r_sb[0:H, :], in_=o1_ar[:, :]),
                   step=16, key=("dma", "h2_in"))
            S.wait("pe", ("dma", "h2_in"))
            S.wait("pe", ("dve", "const"))
            for dc in range(2):
                S.inst("pe", "pe",
                       lambda dc=dc:
                       nc.tensor.transpose(
                           cst[:, dc * H:(dc + 1) * H],
                           o1r_sb[0:H, dc * P:(dc + 1) * P],
                           iden_f[0:H, 0:H]),
                       key=("pe", "h2_tr") if dc == 1 else None)
            S.wait("dve", ("pe", "h2_tr"))
            S.inst("dve", "dve",
                   lambda: nc.vector.tensor_copy(cs2_col[:, :], cst[:, :]),
                   key=("dve", "h2_col"))
        S.wait("pe", ("dma", "W"))
        if tag == "h1":
            S.wait("pe", ("dma", "h1_in"))
        else:
            S.wait("pe", ("dve", "h2_col"))
        for h in range(H):
            for oc in range(2):
                for dc in range(2):
                    widx = (h * 2 + dc) * 2 + oc
                    S.inst("pe", "pe",
                           lambda h=h, oc=oc, dc=dc, widx=widx, col_sb=col_sb:
                           nc.tensor.matmul(
                               ht[:, oc * H + h:oc * H + h + 1],
                               wsb[:, widx * P:(widx + 1) * P],
                               col_sb[:, col_idx(dc, h):col_idx(dc, h) + 1],
                               start=(dc == 0), stop=(dc == 1),
                               skip_group_check=True),
                           key=("pe", tag + "_mm")
                           if (h, oc, dc) == (H - 1, 1, 1) else None)
        S.wait("act", ("pe", tag + "_mm"))
        S.inst("act", "act",
               lambda: nc.scalar.activation(
                   hcol[:, :], ht[:, :], AF.Tanh, scale=1.0 / float(n_total)),
               key=("act", tag))

    def pass_bc(S, tag):
        """Per-tile scores+weighted-sum from resident SBUF copies.
        ACT sign/sigmoid batched per group of G tiles."""
        is_c = tag == "C"
        htag = "h1" if tag == "B" else "h2"

        def grp(t):
            return t // G

        def mms_tile(t):
            g = grp(t)
            b = g % 2
            r = rows_of(t)
            col = (t - g * G) * H
            if t == 0:
                S.wait("pe", ("act", htag))
                if tag == "B":
                    # all resident xts copies must have landed (the ACT-side
                    # ones are covered transitively through h1; wait on the
                    # last DVE-side copy explicitly)
                    last_dve = T - 1 if (T - 1) % 2 == 0 else T - 2
                    if last_dve >= 0:
                        S.wait("pe", ("cp", "A_copy", last_dve))
            if g >= 2 and t == group_tiles[g][0]:
                S.wait("pe", ("act", tag + "_sig", g - 2))
            for oc in range(2):
                S.inst("pe", "pe",
                       lambda t=t, b=b, r=r, col=col, oc=oc:
                       nc.tensor.matmul(
                           spsum[b][0:r, col:col + H],
                           xts[:, t * D + oc * P:t * D + oc * P + r],
                           hcol[:, oc * H:(oc + 1) * H],
                           start=(oc == 0), stop=(oc == 1),
                           skip_group_check=True),
                       key=("pe", tag + "_mmS", t) if oc == 1 else None)

        def mmout_tile(t):
            g = grp(t)
            r = rows_of(t)
            if is_c:
                S.wait("pe", ("dve", "C_w", g))
                lhs = lambda t=t, r=r, g=g: wv[g % 2][0:r, (t - g * G) * H:(t - g * G) * H + H]
            else:
                S.wait("pe", ("act", "B_sig", g))
                lhs = lambda t=t, r=r: att1[0:r, t * H:(t + 1) * H]
            S.inst("pe", "pe",
                   lambda t=t, r=r, lhs=lhs:
                   nc.tensor.matmul(
                       outacc[:, :],
                       lhs(),
                       x_nat[0:r, t * D:(t + 1) * D],
                       start=(t == 0), stop=(t == T - 1),
                       skip_group_check=True),
                   key=("pe", tag + "_mmOut", t))

        def act_group(g):
            b = g % 2
            tiles = group_tiles[g]
            ncols = len(tiles) * H
            rmax = rows_of(tiles[0])
            S.wait("act", ("pe", tag + "_mmS", tiles[-1]))
            if g >= 2:
                S.wait("act", ("act", tag + "_sig", g - 2))
                if is_c:
                    S.wait("act", ("dve", "C_w", g - 2))
            S.inst("act", "act",
                   lambda g=g, b=b, rmax=rmax, ncols=ncols:
                   nc.scalar.activation(
                       sgn[b][0:rmax, 0:ncols], spsum[b][0:rmax, 0:ncols],
                       AF.Sign),
                   key=("act", tag + "_sgn", g))
            S.wait("act", ("act", tag + "_sgn", g))
            if is_c:
                dst = lambda g=g, b=b, rmax=rmax, ncols=ncols: att2[b][0:rmax, 0:ncols]
            else:
                dst = lambda g=g, rmax=rmax, ncols=ncols: \
                    att1[0:rmax, g * G * H:g * G * H + ncols]
            S.inst("act", "act",
                   lambda g=g, b=b, rmax=rmax, ncols=ncols, dst=dst:
                   nc.scalar.activation(dst(), sgn[b][0:rmax, 0:ncols],
                                        AF.Sigmoid),
                   key=("act", tag + "_sig", g))

        def dve_w(g):
            b = g % 2
            tiles = group_tiles[g]
            ncols = len(tiles) * H
            rmax = rows_of(tiles[0])
            S.wait("dve", ("act", "C_sig", g))
            S.inst("dve", "dve",
                   lambda g=g, b=b, rmax=rmax, ncols=ncols:
                   nc.vector.tensor_mul(
                       wv[b][0:rmax, 0:ncols], att2[b][0:rmax, 0:ncols],
                       att1[0:rmax, g * G * H:g * G * H + ncols]),
                   key=("dve", "C_w", g))

        # PE stream: scores for group g+1 interleave with mmOut of group g
        for t in group_tiles[0]:
            mms_tile(t)
        for g in range(n_groups):
            if g + 1 < n_groups:
                for t in group_tiles[g + 1]:
                    mms_tile(t)
            for t in group_tiles[g]:
                mmout_tile(t)
        # ACT stream
        for g in range(n_groups):
            act_group(g)
        # DVE stream
        if is_c:
            for g in range(n_groups):
                dve_w(g)

    def col_idx_h1(dc, h):
        return dc

    def col_idx_h2(dc, h):
        return dc * H + h

    def sched(S):
        # ---- preamble: zero spsum tails (DVE; partial tiles leave rows
        # uninitialized and the grouped sign/sigmoid reads full rectangles)
        S.inst("dve", "dve", lambda: nc.vector.memset(spsum[0].ap(), 0.0))
        S.inst("dve", "dve", lambda: nc.vector.memset(spsum[1].ap(), 0.0))
        # ---- preamble: constants (gpsimd), W loads (sync) ----
        S.inst("pool", "poolc", lambda: nc.gpsimd.memset(ones_f.ap(), 1.0),
               key=("dve", "ones"))
        S.inst("pool", "poolc", lambda: nc.gpsimd.memset(ones_b.ap(), 1.0),
               key=("dve", "ones_b"))
        S.wait("pool", ("dve", "ones"))
        S.wait("pool", ("dve", "ones_b"))
        S.inst("pool", "poolc",
               lambda: nc.gpsimd.affine_select(
                   iden_f.ap(), ones_f.ap(), pattern=[[-1, P]],
                   compare_op=ALU.is_equal, fill=0.0, base=0,
                   channel_multiplier=1),
               key=("dve", "iden_f"))
        S.inst("pool", "poolc",
               lambda: nc.gpsimd.affine_select(
                   iden_b.ap(), ones_b.ap(), pattern=[[-1, P]],
                   compare_op=ALU.is_equal, fill=0.0, base=0,
                   channel_multiplier=1),
               key=("dve", "const"))
        for h in range(H):
            for dc in range(2):
                for oc in range(2):
                    widx = (h * 2 + dc) * 2 + oc
                    S.inst("sp", "dma_w",
                           lambda h=h, dc=dc, oc=oc, widx=widx:
                           nc.sync.dma_start(
                               out=wsb[:, widx * P:(widx + 1) * P],
                               in_=w_ext[h, dc * P:(dc + 1) * P, oc * P:(oc + 1) * P]),
                           step=16,
                           key=("dma", "W") if widx == H * 4 - 1 else None)

        # ---- phase A: chunk loads (pool, SWDGE cast) gated 2-deep ----
        for c in range(n_chunks):
            if c >= 2:
                S.wait("pool", ("pe", "A_chunk", c - 2))
            chunk_load(S, c)
        # ---- phase A: per tile 2 PE transposes; plain copies xtp->xts on
        # DVE (even tiles) / ACT (odd tiles); colsum via per-chunk 3D-AP
        # reduces over the resident xts (DVE), accumulated in cacc.
        if T * P != n_rows:
            # zero the tail tile's unwritten xts columns so chunk reduces
            # can read full rectangles
            tt = T - 1
            r = rows_of(tt)
            for oc in range(2):
                S.inst("dve", "dve",
                       lambda tt=tt, r=r, oc=oc:
                       nc.vector.memset(
                           xts[:, tt * D + oc * P + r:tt * D + (oc + 1) * P], 0.0))
        for c in range(n_chunks):
            S.wait("pe", ("dma", "load", c))
            if c == 0:
                S.wait("pe", ("dve", "const"))
            for t in chunk_tiles[c]:
                r = rows_of(t)
                if t >= 4:
                    S.wait("pe", ("cp", "A_copy", t - 4))
                for oc in range(2):
                    S.inst("pe", "pe",
                           lambda t=t, r=r, oc=oc:
                           nc.tensor.transpose(
                               xtp[t % 4][:, oc * P:oc * P + r],
                               x_nat[0:r, t * D + oc * P:t * D + (oc + 1) * P],
                               iden_b[0:r, 0:r]),
                           key=("pe", "A_tr", t) if oc == 1 else None)
                if t == chunk_tiles[c][-1]:
                    if S.plan is None:
                        S.ev[("pe", "A_chunk", c)] = S.ev[("pe", "A_tr", t)]
            # copies for this chunk (one instruction per full tile; the
            # partial tail tile keeps restricted per-half copies so it
            # doesn't clobber the zeroed pad columns)
            for t in chunk_tiles[c]:
                r = rows_of(t)
                eng, sem = ("dve", "dve") if t % 2 == 0 else ("act", "act")
                S.wait(eng, ("pe", "A_tr", t))
                if r == P:
                    if t % 2 == 0:
                        S.inst(eng, sem,
                               lambda t=t:
                               nc.vector.tensor_copy(
                                   xts[:, t * D:(t + 1) * D],
                                   xtp[t % 4][:, :]),
                               key=("cp", "A_copy", t))
                    else:
                        S.inst(eng, sem,
                               lambda t=t:
                               nc.scalar.copy(
                                   xts[:, t * D:(t + 1) * D],
                                   xtp[t % 4][:, :]),
                               key=("cp", "A_copy", t))
                else:
                    for oc in range(2):
                        if t % 2 == 0:
                            S.inst(eng, sem,
                                   lambda t=t, r=r, oc=oc:
                                   nc.vector.tensor_copy(
                                       xts[:, t * D + oc * P:t * D + oc * P + r],
                                       xtp[t % 4][:, oc * P:oc * P + r]),
                                   key=("cp", "A_copy", t) if oc == 1 else None)
                        else:
                            S.inst(eng, sem,
                                   lambda t=t, r=r, oc=oc:
                                   nc.scalar.copy(
                                       xts[:, t * D + oc * P:t * D + oc * P + r],
                                       xtp[t % 4][:, oc * P:oc * P + r]),
                                   key=("cp", "A_copy", t) if oc == 1 else None)
            # chunk colsum partials from resident xts (both halves)
            last_odd_c = max((t for t in chunk_tiles[c] if t % 2 == 1),
                             default=None)
            last_even_c = max((t for t in chunk_tiles[c] if t % 2 == 0),
                              default=None)
            if last_odd_c is not None:
                S.wait("dve", ("cp", "A_copy", last_odd_c))
            if last_even_c is not None:
                S.wait("dve", ("cp", "A_copy", last_even_c))
            base = chunk_tiles[c][0] * D
            nt = len(chunk_tiles[c])
            for oc in range(2):
                S.inst("dve", "dve",
                       lambda c=c, base=base, nt=nt, oc=oc:
                       nc.vector.tensor_reduce(
                           cacc[:, oc * n_chunks + c:oc * n_chunks + c + 1],
                           xts.ap().rearrange("p (t m) -> p t m", m=D)
                               [:, chunk_tiles[c][0]:chunk_tiles[c][0] + nt,
                                oc * P:(oc + 1) * P],
                           axis=mybir.AxisListType.XY,
                           op=ALU.add),
                       key=("dve", "A_red", c) if oc == 1 else None)
        # final colsum reduction (cacc -> cs_col)
        S.wait("dve", ("dve", "A_red", n_chunks - 1))
        for dc in range(2):
            S.inst("dve", "dve",
                   lambda dc=dc:
                   nc.vector.reduce_sum(
                       cs_col[:, dc:dc + 1],
                       cacc[:, dc * n_chunks:dc * n_chunks + n_chunks],
                       axis=mybir.AxisListType.X),
                   key=("dve", "cs_red") if dc == 1 else None)
        # colsum -> AR
        S.wait("sp", ("dve", "cs_red"))
        S.inst("sp", "dma_m",
               lambda: nc.sync.dma_start(out=cs_dram[:, :], in_=cs_col[:, :]),
               step=16, key=("dma", "cs_out"))
        S.wait("pool", ("dma", "cs_out"))
        S.inst("pool", "cc",
               lambda: nc.gpsimd.collective_compute(
                   "AllReduce", mybir.AluOpType.add,
                   replica_groups=[list(range(n_cores))],
                   ins=[cs_dram[:, :]], outs=[cs_ar[:, :]]),
               key=("cc", "ar1"))
        h_phase(S, "h1", ("cc", "ar1"), cs_col, col_idx_h1)
        # ---- phase B ----
        pass_bc(S, "B")
        # out1 -> AR
        S.wait("act", ("pe", "B_mmOut", T - 1))
        S.inst("act", "act",
               lambda: nc.scalar.copy(o1_sb[0:H, :], outacc[:, :]),
               key=("act", "o1_copy"))
        S.wait("sp", ("act", "o1_copy"))
        S.inst("sp", "dma_m",
               lambda: nc.sync.dma_start(out=o1_dram[:, :], in_=o1_sb[0:H, :]),
               step=16, key=("dma", "o1_out"))
        S.wait("pool", ("dma", "o1_out"))
        S.inst("pool", "cc",
               lambda: nc.gpsimd.collective_compute(
                   "AllReduce", mybir.AluOpType.add,
                   replica_groups=[list(range(n_cores))],
                   ins=[o1_dram[:, :]], outs=[o1_ar[:, :]]),
               key=("cc", "ar2"))
        h_phase(S, "h2", ("cc", "ar2"), cs2_col, col_idx_h2)
        # ---- phase C ----
        pass_bc(S, "C")
        # final out (stage through o1_sb)
        S.wait("act", ("pe", "C_mmOut", T - 1))
        S.inst("act", "act",
               lambda: nc.scalar.copy(o1_sb[0:H, :], outacc[:, :]),
               key=("act", "out_copy"))
        S.wait("sp", ("act", "out_copy"))
        S.inst("sp", "dma_m",
               lambda: nc.sync.dma_start(out=out_ext[:, :], in_=o1_sb[0:H, :]),
               step=16, key=("dma", "out_final"))
        S.wait("sp", ("dma", "out_final"))

    plan = Sched()
    sched(plan)
    emit = Sched(plan.ev)
    sched(emit)

    eng_map = {
        "sp": nc.sync, "pe": nc.tensor, "act": nc.scalar,
        "dve": nc.vector, "pool": nc.gpsimd,
    }

    def run_ops(eng_name):
        eng = eng_map[eng_name]
        def body(_engine):
            for op in emit.ops[eng_name]:
                if op[0] == "w":
                    _, sem, v = op
                    eng.wait_ge(sems[sem], v)
                else:
                    _, thunk, sem, step = op
                    bi = thunk()
                    bi.then_inc(sems[sem], step)
        return body

    with nc.Block() as block:
        block.sync(run_ops("sp"))
        block.gpsimd(run_ops("pool"))
        block.tensor(run_ops("pe"))
        block.scalar(run_ops("act"))
        block.vector(run_ops("dve"))

    return nc


_NC_CACHE = {}


def _get_nc(n_rows, n_cores, n_total):
    key = (n_rows, n_cores, n_total)
    if key not in _NC_CACHE:
        _NC_CACHE[key] = build_kernel(n_rows, n_cores, n_total)
    return _NC_CACHE[key]


def kernel(x, W):
    from concourse.bass_utils import run_bass_kernel_spmd

    x = np.ascontiguousarray(np.asarray(x, dtype=np.float32))
    W = np.ascontiguousarray(np.asarray(W, dtype=np.float32))
    n, d = x.shape
    assert n % N_CORES == 0 and d == D
    n_rows = n // N_CORES

    nc = _get_nc(n_rows, N_CORES, n)
    in_maps = [
        {"x": x[i * n_rows:(i + 1) * n_rows], "W": W} for i in range(N_CORES)
    ]
    res = run_bass_kernel_spmd(nc, in_maps, core_ids=list(range(N_CORES)))
    total = np.zeros((H, D), dtype=np.float64)
    for i in range(N_CORES):
        total += res.results[i]["out"].astype(np.float64)
    return total.astype(np.float32).reshape(1, H * D)


if __name__ == "__main__":
    rng = np.random.default_rng(0)
    x = rng.standard_normal((N_TOTAL, D)).astype(np.float32)
    W = (rng.standard_normal((H, D, D)) * np.sqrt(2.0 / (D + D))).astype(np.float32)
    out = kernel(x=x, W=W)
    print(out.shape, out[0, :4])
